# revision 46
# baseline (speedup 1.0000x reference)
"""FCOS detection-head kernel for Trainium2 (8 NeuronCores, batch-parallel).

Strategy
--------
- Data-parallel over batch: core i processes image i (BATCH=8, n_cores=8).
- Each 3x3 conv is 18 accumulating PE matmuls per output tile
  (9 taps x 2 input-channel chunks of 128), dtype float32r (fp32 bytes,
  full PE rate for moving dim >= 256).
- Activations tiled by padded rows; each tile loads its own halo rows from
  DRAM (implicit zero padding via a memset tile).
- GroupNorm: per-channel sum fused into the PSUM drain (ScalarE activation
  accum_out), sumsq via a Square activation; group reduce + broadcast via
  tiny indicator matmuls on the PE; conv bias folded analytically into the
  group statistics; scale/shift+ReLU applied on the consumer's input load.
- Heads: cls (M=20) and reg+ctr packed (M=5); outputs PE-transposed to the
  NHWC flatten layout the reference produces.
"""

import os
import sys
from contextlib import ExitStack

import numpy as np

for _p in ("/opt/trn_rl_repo", "/root/.axon_site/_ro/trn_rl_repo"):
    if os.path.isdir(_p) and _p not in sys.path:
        sys.path.insert(0, _p)

import concourse.bacc as bacc
import concourse.bass as bass
import concourse.tile as tile
from concourse import mybir
from concourse.bass_utils import run_bass_kernel_spmd

F32 = mybir.dt.float32
F32R = mybir.dt.float32r
AF = mybir.ActivationFunctionType
AX = mybir.AxisListType
ALU = mybir.AluOpType

C = 256
NCLS = 20
GROUPS = 16
EPS = 1e-5
BATCH = 8
LEVELS = [(100, 128), (50, 64), (25, 32)]
RTILE = [3, 7, 15]  # output rows per conv tile (N = R*(W+2) <= 512)
LVL_BASE = [0, 12800, 16000]
HW_TOT = 16800


def _tiles_for(H, R):
    """Split H rows into tiles of <=R rows, keeping every tile >=2 rows so
    the f32r matmul moving dim stays >=256."""
    out = []
    r = 0
    while r < H:
        rr = min(R, H - r)
        if rr < 2 and out:  # steal a row from the previous tile
            pr, prr = out.pop()
            out.append((pr, prr - 1))
            r -= 1
            rr = min(R, H - r)
        out.append((r, rr))
        r += rr
    return out


def build_program(levels=(0, 1, 2), repeat=1):
    nc = bacc.Bacc("TRN2", target_bir_lowering=False, debug=False,
                   num_devices=BATCH)

    feats = {l: nc.dram_tensor(f"feat{l}", [C, *LEVELS[l]], F32,
                               kind="ExternalInput") for l in levels}
    wt = {n: nc.dram_tensor(n, [128, 2 * 9 * C], F32, kind="ExternalInput")
          for n in ("w_cls1", "w_cls2", "w_reg1", "w_reg2")}
    w_clsO = nc.dram_tensor("w_clsO", [128, 2 * 9 * NCLS], F32,
                            kind="ExternalInput")
    w_regO = nc.dram_tensor("w_regO", [128, 2 * 9 * 5], F32,
                            kind="ExternalInput")
    packs = {}
    for lay in ("cls1", "cls2", "reg1", "reg2"):
        for p in ("gamma", "beta", "bias", "bias2"):
            packs[(p, lay)] = nc.dram_tensor(f"{p}_{lay}", [128, 2], F32,
                                             kind="ExternalInput")
    b_clsO = nc.dram_tensor("b_clsO", [NCLS, 1], F32, kind="ExternalInput")
    b_regO = nc.dram_tensor("b_regO", [5, 1], F32, kind="ExternalInput")
    gid_d = nc.dram_tensor("gid", [128, 8], F32, kind="ExternalInput")
    gbc_d = nc.dram_tensor("gbc", [8, 128], F32, kind="ExternalInput")
    idn_d = nc.dram_tensor("idn", [32, 32], F32, kind="ExternalInput")

    ys = {}
    for tw in ("c", "r"):
        for st in (1, 2):
            for l in levels:
                H, W = LEVELS[l]
                # stored with 2 zero pad columns at the end of each row so
                # consumer tiles load one contiguous flat run per chunk that
                # already contains the conv zero-padding between rows
                ys[(tw, st, l)] = nc.dram_tensor(
                    f"y_{tw}{st}_{l}", [2, 128, H, W + 2], F32,
                    kind="Internal")

    out_cls = nc.dram_tensor("out_cls", [HW_TOT, NCLS], F32,
                             kind="ExternalOutput")
    out_reg = nc.dram_tensor("out_reg", [HW_TOT, 4], F32,
                             kind="ExternalOutput")
    out_ctr = nc.dram_tensor("out_ctr", [1, HW_TOT], F32,
                             kind="ExternalOutput")

    with ExitStack() as ctx:
        tc = ctx.enter_context(tile.TileContext(nc))
        xpool = ctx.enter_context(tc.tile_pool(name="xin", bufs=6))
        wpool = ctx.enter_context(tc.tile_pool(name="wts", bufs=2))
        whpool = ctx.enter_context(tc.tile_pool(name="wh", bufs=2))
        cpool = ctx.enter_context(tc.tile_pool(name="consts", bufs=1))
        pspool = ctx.enter_context(tc.tile_pool(name="cps", bufs=3,
                                                space="PSUM"))
        tppool = ctx.enter_context(tc.tile_pool(name="tps", bufs=2,
                                                space="PSUM"))
        gpspool = ctx.enter_context(tc.tile_pool(name="gps", bufs=2,
                                                 space="PSUM"))
        stpool = ctx.enter_context(tc.tile_pool(name="stage", bufs=4))
        sqpool = ctx.enter_context(tc.tile_pool(name="sq", bufs=2))
        spool = ctx.enter_context(tc.tile_pool(name="stats", bufs=6))
        gsmall = ctx.enter_context(tc.tile_pool(name="gsmall", bufs=12))
        gout = ctx.enter_context(tc.tile_pool(name="gnout", bufs=16))
        ospool = ctx.enter_context(tc.tile_pool(name="ostage", bufs=4))

        # ---- constants and params to SBUF ----
        gid_sb = cpool.tile([128, 8], F32, tag="c_gid")
        nc.sync.dma_start(out=gid_sb[:], in_=gid_d[:])
        gbc_sb = cpool.tile([8, 128], F32, tag="c_gbc")
        nc.sync.dma_start(out=gbc_sb[:], in_=gbc_d[:])
        idn_sb = cpool.tile([32, 32], F32, tag="c_idn")
        nc.sync.dma_start(out=idn_sb[:], in_=idn_d[:])
        eps_sb = cpool.tile([128, 1], F32, tag="c_eps")
        nc.vector.memset(eps_sb[:], EPS)
        bclsO_sb = cpool.tile([NCLS, 1], F32, tag="c_bcls")
        nc.sync.dma_start(out=bclsO_sb[:], in_=b_clsO[:])
        bregO_sb = cpool.tile([5, 1], F32, tag="c_breg")
        nc.sync.dma_start(out=bregO_sb[:], in_=b_regO[:])
        zsrc_sb = cpool.tile([128, 1024], F32, tag="c_zeros")
        nc.vector.memset(zsrc_sb[:], 0.0)
        psb = {}
        for lay in ("cls1", "cls2", "reg1", "reg2"):
            for p in ("gamma", "beta", "bias", "bias2"):
                t = cpool.tile([128, 2], F32, tag=f"c_{p}_{lay}")
                nc.sync.dma_start(out=t[:], in_=packs[(p, lay)][:])
                psb[(p, lay)] = t

        def load_w(dram, cols, tag, pool):
            t = pool.tile([128, cols], F32R, tag=tag)
            nc.gpsimd.dma_start(out=t[:], in_=dram[:])
            return t

        def zero(out_ap):
            """Zero an AP by copying from a known-zero tile (memset on
            float32r fails the walrus ISA check, and scale=0 tricks
            propagate NaN from uninitialized SBUF)."""
            dims = out_ap.shape[1:]
            n = 1
            for d in dims:
                n *= d
            src = zsrc_sb[:, 0:n]
            if len(dims) == 2:
                src = src.rearrange("p (a b) -> p a b", a=dims[0])
            elif len(dims) == 3:
                src = src.rearrange("p (a b c) -> p a b c", a=dims[0],
                                    b=dims[1])
            nc.scalar.activation(out=out_ap, in_=src, func=AF.Copy)

        # ---- shared tile-level helpers ----
        def load_xin(l, src_ap_fn, apply_ss, or0, rr, padded):
            """Allocate + fill one padded input tile. Returns (xin, F, Wp).

            padded=True: source rows are already W+2 wide with zero pad
            columns, so each chunk is one contiguous flat DMA and only the
            leading/trailing pad elements need zeroing.
            """
            H, W = LEVELS[l]
            Wp = W + 2
            R = RTILE[l]
            F = (R + 2) * Wp + 2
            xin = xpool.tile([128, 2, F], F32R, tag="xin")
            rlo = max(or0 - 1, 0)
            rhi = min(or0 + rr + 1, H)
            jlo = rlo - (or0 - 1)
            nrow = rhi - rlo

            if padded:
                zero(xin[:, :, 0: 2 + jlo * Wp])      # lead + top rows
                end = 2 + (jlo + nrow) * Wp
                if end < F:
                    zero(xin[:, :, end: F])           # bottom rows + tail
                for k in (0, 1):
                    nc.gpsimd.dma_start(
                        out=xin[:, k, 2 + jlo * Wp: end],
                        in_=src_ap_fn(k, rlo, rhi))
            else:
                grid = xin[:, :, 1: 1 + (R + 2) * Wp].rearrange(
                    "p k (r w) -> p k r w", w=Wp)
                zero(grid[:, :, :, 0:1])          # left pad col
                zero(grid[:, :, :, W + 1:W + 2])  # right pad col
                zero(xin[:, :, 0:1])              # leading pad element
                zero(xin[:, :, F - 1:F])          # trailing pad element
                if jlo > 0:                       # top zero row(s)
                    zero(grid[:, :, 0:jlo, :])
                if jlo + nrow < R + 2:            # bottom zero rows
                    zero(grid[:, :, jlo + nrow: R + 2, :])
                for k in (0, 1):
                    dstv = xin[:, k, 2 + jlo * Wp: 2 + (jlo + nrow) * Wp] \
                        .rearrange("p (r w) -> p r w", w=Wp)[:, :, 0:W]
                    nc.gpsimd.dma_start(out=dstv, in_=src_ap_fn(k, rlo, rhi))
            if apply_ss is not None:
                sc, sh = apply_ss
                for k in (0, 1):
                    iv = xin[:, k, 2 + jlo * Wp: 2 + (jlo + nrow) * Wp] \
                        .rearrange("p (r w) -> p r w", w=Wp)[:, :, 0:W]
                    nc.scalar.activation(out=iv, in_=iv, func=AF.Relu,
                                         bias=sh[:, k: k + 1],
                                         scale=sc[:, k: k + 1])
            return xin, F, Wp

        def mm_accum(psN, xin, w_sb, Wp, N, mc, M):
            i = 0
            for k in (0, 1):
                for tap in range(9):
                    dy, dx = tap // 3, tap % 3
                    off = dy * Wp + dx
                    rhs = xin[:, k, off: off + N]
                    base = (k * 9 + tap) * M + mc * 128
                    mwid = 128 if M > 128 else M
                    lhsT = w_sb[:, base: base + mwid]
                    nc.tensor.matmul(psN, lhsT, rhs,
                                     start=(i == 0), stop=(i == 17))
                    i += 1

        # ---- tower conv pass (256 -> 256, collects GN stats) ----
        def conv_pass(l, src_ap_fn, apply_ss, w_sb, out_y, padded):
            H, W = LEVELS[l]
            Wp = W + 2
            tiles = _tiles_for(H, RTILE[l])
            T = len(tiles)
            S = spool.tile([128, 2, 34], F32, tag="statS")
            Q = spool.tile([128, 2, 34], F32, tag="statQ")
            for t, (or0, rr) in enumerate(tiles):
                xin, F, _ = load_xin(l, src_ap_fn, apply_ss, or0, rr, padded)
                N = rr * Wp
                for mc in (0, 1):
                    ps = pspool.tile([128, 512], F32, tag="cps")
                    mm_accum(ps[:, 0:N], xin, w_sb, Wp, N, mc, C)
                    # stage carries the 2 zero pad columns per row so the y
                    # store is one contiguous run per partition
                    stage = stpool.tile([128, rr * Wp], F32, tag="stage")
                    sgrid = stage.rearrange("p (r w) -> p r w", w=Wp)
                    stv = sgrid[:, :, 0:W]
                    src = ps[:, 0:N].rearrange("p (r w) -> p r w",
                                               w=Wp)[:, :, 1:W + 1]
                    nc.scalar.activation(out=stv, in_=src, func=AF.Copy,
                                         accum_out=S[:, mc, t: t + 1])
                    zero(sgrid[:, :, W:Wp])
                    sq = sqpool.tile([128, rr * W], F32, tag="sq")
                    nc.scalar.activation(
                        out=sq.rearrange("p (r w) -> p r w", w=W), in_=stv,
                        func=AF.Square, accum_out=Q[:, mc, t: t + 1])
                    nc.sync.dma_start(
                        out=out_y[mc, :, or0: or0 + rr, :].rearrange(
                            "p h w -> p (h w)"),
                        in_=stage[:])
            return S, Q, T

        # ---- GN statistics finalize -> (scale, shift) [128,2] ----
        def finalize(lay, l, S, Q, T):
            H, W = LEVELS[l]
            m = float(H * W)
            n = float(GROUPS // 16 * 16 * H * W)  # 16 channels per group
            biasp = psb[("bias", lay)]
            bias2p = psb[("bias2", lay)]
            gammap = psb[("gamma", lay)]
            betap = psb[("beta", lay)]
            rhs = gsmall.tile([128, 10], F32, tag="grhs")
            for k in (0, 1):
                b = 5 * k
                nc.vector.tensor_reduce(out=rhs[:, b: b + 1],
                                        in_=S[:, k, 0:T], axis=AX.X,
                                        op=ALU.add)
                nc.vector.tensor_reduce(out=rhs[:, b + 1: b + 2],
                                        in_=Q[:, k, 0:T], axis=AX.X,
                                        op=ALU.add)
                nc.vector.tensor_mul(rhs[:, b + 2: b + 3], rhs[:, b: b + 1],
                                     biasp[:, k: k + 1])
                nc.vector.tensor_scalar(out=rhs[:, b + 3: b + 4],
                                        in0=biasp[:, k: k + 1], scalar1=m,
                                        scalar2=None, op0=ALU.mult)
                nc.vector.tensor_scalar(out=rhs[:, b + 4: b + 5],
                                        in0=bias2p[:, k: k + 1], scalar1=m,
                                        scalar2=None, op0=ALU.mult)
            gps = gpspool.tile([128, 16], F32, tag="gps")
            nc.tensor.matmul(gps[0:8, 0:10], gid_sb[:], rhs[:],
                             start=True, stop=True)
            gst = gsmall.tile([8, 10], F32, tag="gst")
            nc.vector.tensor_copy(out=gst[:], in_=gps[0:8, 0:10])
            gv = gst.rearrange("p (k c) -> p c k", c=5)  # [8, col, chunk]
            bc = gsmall.tile([8, 4], F32, tag="gbcr")
            bcv = bc.rearrange("p (k c) -> p c k", c=2)
            t1 = gsmall.tile([8, 2], F32, tag="gt1")
            nc.vector.tensor_add(t1[:], gv[:, 0, :], gv[:, 3, :])
            nc.vector.tensor_scalar(out=bcv[:, 0, :], in0=t1[:],
                                    scalar1=1.0 / n, scalar2=None,
                                    op0=ALU.mult)
            t2 = gsmall.tile([8, 2], F32, tag="gt2")
            nc.vector.tensor_scalar(out=t2[:], in0=gv[:, 2, :], scalar1=2.0,
                                    scalar2=None, op0=ALU.mult)
            nc.vector.tensor_add(t2[:], t2[:], gv[:, 1, :])
            nc.vector.tensor_add(t2[:], t2[:], gv[:, 4, :])
            nc.vector.tensor_scalar(out=t2[:], in0=t2[:], scalar1=1.0 / n,
                                    scalar2=None, op0=ALU.mult)
            t3 = gsmall.tile([8, 2], F32, tag="gt3")
            nc.vector.tensor_mul(t3[:], bcv[:, 0, :], bcv[:, 0, :])
            nc.vector.tensor_sub(t2[:], t2[:], t3[:])
            nc.scalar.activation(out=t2[:], in_=t2[:], func=AF.Sqrt,
                                 bias=eps_sb[0:8], scale=1.0)
            nc.vector.reciprocal(out=bcv[:, 1, :], in_=t2[:])
            gbp = gpspool.tile([128, 16], F32, tag="gps")
            nc.tensor.matmul(gbp[:, 0:4], gbc_sb[:], bc[:],
                             start=True, stop=True)
            mb = gsmall.tile([128, 4], F32, tag="gmb")
            nc.vector.tensor_copy(out=mb[:], in_=gbp[:, 0:4])
            mbv = mb.rearrange("p (k c) -> p c k", c=2)
            sc = gout.tile([128, 2], F32, tag="gsc")
            sh = gout.tile([128, 2], F32, tag="gsh")
            nc.vector.tensor_mul(sc[:], gammap[:], mbv[:, 1, :])
            t4 = gsmall.tile([128, 2], F32, tag="gt4")
            nc.vector.tensor_sub(t4[:], biasp[:], mbv[:, 0, :])
            nc.vector.tensor_mul(t4[:], sc[:], t4[:])
            nc.vector.tensor_add(sh[:], betap[:], t4[:])
            return sc, sh

        # ---- head pass (cls: M=20; reg+ctr packed: M=5) ----
        def head_pass(l, src_ap_fn, apply_ss, w_sb, kind):
            H, W = LEVELS[l]
            Wp = W + 2
            tiles = _tiles_for(H, RTILE[l])
            base = LVL_BASE[l]
            M = NCLS if kind == "cls" else 5
            Mt = NCLS if kind == "cls" else 4  # transposed columns
            for t, (or0, rr) in enumerate(tiles):
                xin, F, _ = load_xin(l, src_ap_fn, apply_ss, or0, rr, True)
                N = rr * Wp
                ps = pspool.tile([128, 512], F32, tag="cps")
                mm_accum(ps[0:M, 0:N], xin, w_sb, Wp, N, 0, M)
                stage = stpool.tile([M, rr * W], F32, tag="hstage")
                stv = stage.rearrange("p (r w) -> p r w", w=W)
                src = ps[0:M, 0:N].rearrange("p (r w) -> p r w",
                                             w=Wp)[:, :, 1:W + 1]
                if kind == "cls":
                    nc.scalar.activation(out=stv, in_=src, func=AF.Identity,
                                         bias=bclsO_sb[:, 0:1], scale=1.0)
                    tsrc = stage
                else:
                    # raw (bias-added) copy of all 5 rows; then ReLU the 4
                    # bbox rows into a second tile (engines need 32-aligned
                    # partition bases, so rows can't be split at the PSUM).
                    nc.scalar.activation(out=stv, in_=src, func=AF.Identity,
                                         bias=bregO_sb[:, 0:1], scale=1.0)
                    strl = stpool.tile([5, rr * W], F32, tag="hstage2")
                    nc.vector.tensor_scalar_max(strl[0:4, :],
                                                stage[0:4, :], 0.0)
                    tsrc = strl
                ost = ospool.tile([128, rr, Mt], F32, tag="ostage")
                for r in range(rr):
                    pt = tppool.tile([128, 32], F32, tag="tps")
                    nc.tensor.transpose(out=pt[0:W, 0:Mt],
                                        in_=tsrc[0:Mt,
                                                 r * W: (r + 1) * W],
                                        identity=idn_sb[0:Mt, 0:Mt])
                    nc.vector.tensor_copy(out=ost[0:W, r, :],
                                          in_=pt[0:W, 0:Mt])
                r0 = base + or0 * W
                r1 = base + (or0 + rr) * W
                if kind == "cls":
                    dst = out_cls[r0:r1, :].rearrange("(r w) c -> w r c", w=W)
                    nc.sync.dma_start(out=dst, in_=ost[0:W, :, :])
                else:
                    dst = out_reg[r0:r1, :].rearrange("(r w) c -> w r c", w=W)
                    nc.sync.dma_start(out=dst, in_=ost[0:W, :, :])
                    nc.sync.dma_start(out=out_ctr[0:1, r0:r1],
                                      in_=stage[4:5, :])

        # ---- source AP builders ----
        def feat_src(l):
            def fn(k, rlo, rhi):
                return feats[l][k * 128:(k + 1) * 128, rlo:rhi, :]
            return fn

        def y_src(key):
            def fn(k, rlo, rhi):
                return ys[key][k, :, rlo:rhi, :].rearrange(
                    "p h w -> p (h w)")
            return fn



        # ================= emission =================
        for _rep in range(repeat):
            w1c = load_w(wt["w_cls1"], 2 * 9 * C, "wts", wpool)
            w1r = load_w(wt["w_reg1"], 2 * 9 * C, "wts", wpool)

            ss = {}
            # Phase A: first tower layer from raw features
            for l in levels:
                SQ = conv_pass(l, feat_src(l), None, w1c, ys[("c", 1, l)],
                               False)
                ss[("cls1", l)] = finalize("cls1", l, *SQ)
                SQ = conv_pass(l, feat_src(l), None, w1r, ys[("r", 1, l)],
                               False)
                ss[("reg1", l)] = finalize("reg1", l, *SQ)

            # Phase B: second tower layer
            w2c = load_w(wt["w_cls2"], 2 * 9 * C, "wts", wpool)
            w2r = load_w(wt["w_reg2"], 2 * 9 * C, "wts", wpool)
            for l in levels:
                SQ = conv_pass(l, y_src(("c", 1, l)), ss[("cls1", l)], w2c,
                               ys[("c", 2, l)], True)
                ss[("cls2", l)] = finalize("cls2", l, *SQ)
                SQ = conv_pass(l, y_src(("r", 1, l)), ss[("reg1", l)], w2r,
                               ys[("r", 2, l)], True)
                ss[("reg2", l)] = finalize("reg2", l, *SQ)

            # Phase C: heads
            whc = load_w(w_clsO, 2 * 9 * NCLS, "wh", whpool)
            whr = load_w(w_regO, 2 * 9 * 5, "wh", whpool)
            for l in levels:
                head_pass(l, y_src(("c", 2, l)), ss[("cls2", l)], whc, "cls")
                head_pass(l, y_src(("r", 2, l)), ss[("reg2", l)], whr, "reg")

    nc.compile()
    return nc


# ---------------- host side ----------------

def _prep_w(w):
    """[O, I, 3, 3] -> [128, 2*9*O] laid out as (p, (kc, ky, kx, o))."""
    O = w.shape[0]
    a = np.asarray(w, np.float32).transpose(1, 2, 3, 0)      # [I, ky, kx, O]
    a = a.reshape(2, 128, 3, 3, O).transpose(1, 0, 2, 3, 4)  # [p, kc, ky, kx, O]
    return np.ascontiguousarray(a.reshape(128, 2 * 9 * O))


def _pack(v):
    """[256] -> [128, 2] (column per channel chunk)."""
    return np.ascontiguousarray(np.asarray(v, np.float32).reshape(2, 128).T)


_PROG_CACHE = {}


def _get_program(levels=(0, 1, 2), repeat=1):
    key = (tuple(levels), repeat)
    if key not in _PROG_CACHE:
        _PROG_CACHE[key] = build_program(levels, repeat)
    return _PROG_CACHE[key]


def _make_inmaps(inputs, levels=(0, 1, 2)):
    gid = np.zeros((128, 8), np.float32)
    gid[np.arange(128), np.arange(128) // 16] = 1.0
    shared = {
        "w_cls1": _prep_w(inputs["cls_conv_w"][0]),
        "w_cls2": _prep_w(inputs["cls_conv_w"][1]),
        "w_reg1": _prep_w(inputs["reg_conv_w"][0]),
        "w_reg2": _prep_w(inputs["reg_conv_w"][1]),
        "w_clsO": _prep_w(inputs["cls_out_w"]),
        "w_regO": _prep_w(np.concatenate([np.asarray(inputs["reg_out_w"]),
                                          np.asarray(inputs["ctr_w"])], 0)),
        "b_clsO": np.asarray(inputs["cls_out_b"],
                             np.float32).reshape(NCLS, 1),
        "b_regO": np.concatenate([np.asarray(inputs["reg_out_b"]),
                                  np.asarray(inputs["ctr_b"])]
                                 ).astype(np.float32).reshape(5, 1),
        "gid": gid,
        "gbc": np.ascontiguousarray(gid.T),
        "idn": np.eye(32, dtype=np.float32),
    }
    for lay, gk, bk, ck in (("cls1", "cls_gn_g", "cls_gn_b", "cls_conv_b"),
                            ("cls2", "cls_gn_g", "cls_gn_b", "cls_conv_b"),
                            ("reg1", "reg_gn_g", "reg_gn_b", "reg_conv_b"),
                            ("reg2", "reg_gn_g", "reg_gn_b", "reg_conv_b")):
        i = 0 if lay.endswith("1") else 1
        b = np.asarray(inputs[ck][i], np.float32)
        shared[f"gamma_{lay}"] = _pack(inputs[gk][i])
        shared[f"beta_{lay}"] = _pack(inputs[bk][i])
        shared[f"bias_{lay}"] = _pack(b)
        shared[f"bias2_{lay}"] = _pack(b * b)

    in_maps = []
    for i in range(BATCH):
        m = dict(shared)
        for l in levels:
            m[f"feat{l}"] = np.ascontiguousarray(
                np.asarray(inputs[f"feat{l}"][i], np.float32))
        in_maps.append(m)
    return in_maps


def run(inputs, levels=(0, 1, 2), **kw):
    nc = _get_program(levels)
    in_maps = _make_inmaps(inputs, levels)
    res = run_bass_kernel_spmd(nc, in_maps, list(range(BATCH)), **kw)
    cls = np.stack([res.results[i]["out_cls"] for i in range(BATCH)])
    reg = np.stack([res.results[i]["out_reg"] for i in range(BATCH)])
    ctr = np.stack([res.results[i]["out_ctr"] for i in range(BATCH)])
    cls = cls.reshape(BATCH, HW_TOT, NCLS)
    reg = reg.reshape(BATCH, HW_TOT * 4, 1)
    ctr = ctr.reshape(BATCH, HW_TOT, 1)
    return (cls, reg, ctr), res


def kernel(**inputs):
    (cls, reg, ctr), _ = run(inputs)
    return cls, reg, ctr
